# revision 16
# baseline (speedup 1.0000x reference)
"""Trainium2 Bass kernel: 16-head causal attention with sink logit.

Contract: kernel(**inputs) takes the FULL inputs of the reference
(x [2,2048,1024], W_Q/W_K/W_V/W_out [1024,1024], sink [16]) and returns
the FULL output [2,2048,1024], running on 8 NeuronCores.

Sharding: core c = b*4 + g handles batch b and heads [4g, 4g+4).
Each core computes yT_partial [1024, 2048] = W_out_slice^T @ attn^T;
host sums the 4 partials per batch and transposes.

v7 (over v6): normalization broadcast moved off the PE/scalar engines
onto the idle GPSIMD engine (partition_broadcast ISA op); the score->PV
software pipeline deepened (pend 8) so PV of the next head pair never
waits on the previous pair's PSUM drain; vp value layout widened to
128 columns per head (FWL-eligible LDWEIGHTS); yt/K/Q PSUM->SBUF
copies split in half to cut DVE head-of-line blocking.

v8: HW runs sub-128-contraction matmuls at HALF the column rate, so
score matmuls (D=64) are zero-padded to full 128-row contraction: Q is
stored per-head with the other head's 64 rows zeroed (qt2), the K tile
keeps both heads' rows and serves as one shared stationary, and the
two per-head score matmuls fuse into a single matmul over a 2-chunk
moving AP. The sink close matmuls are likewise padded to K=128.
"""

import sys
import numpy as np

if "/opt/trn_rl_repo" not in sys.path:
    sys.path.insert(0, "/opt/trn_rl_repo")

B, T, C = 2, 2048, 1024
H, D = 16, 64
G = 4                # heads per core
DH = G * D           # 256 head-dims per core
NCORES = 8
QC = 512             # q chunk (matmul moving free dim)
NQ = T // QC         # 4
NKT = T // 128       # 16 k-tiles
NCC = C // 128       # 8 contraction chunks over C
SCALE = 1.0 / float(np.sqrt(D))
PEND = 8             # score->PV pipeline depth (in k-tiles)

# vp_sb per-kt slot layout (512 cols per kt), all heads 128 wide:
#   head0 (even): [V(64) | one | zeros(63)]  off 0,   denom row 64
#   head1 (odd):  [one | zeros(63) | V(64)]  off 128, denom row 0
#   head2 (even): [V(64) | one | zeros(63)]  off 256, denom row 64
#   head3 (odd):  [one | zeros(63) | V(64)]  off 384, denom row 0
VP_W = 512
VP_OFF = [0, 128, 256, 384]


def build_program(reps=1):
    """Build the per-core Bass program. reps>1 repeats the compute body
    (same inputs -> same outputs) for differential wall-clock timing."""
    from contextlib import ExitStack

    import concourse.bass as bass
    import concourse.tile as tile
    from concourse import bacc, mybir

    f32 = mybir.dt.float32
    bf16 = mybir.dt.bfloat16
    AF = mybir.ActivationFunctionType

    nc = bacc.Bacc("TRN2", target_bir_lowering=False, debug=False)

    xt_d = nc.dram_tensor("xt", [C, T], bf16, kind="ExternalInput").ap()
    wq_d = nc.dram_tensor("wq", [C, DH], bf16, kind="ExternalInput").ap()
    wk_d = nc.dram_tensor("wk", [C, DH], bf16, kind="ExternalInput").ap()
    wv_d = nc.dram_tensor("wv", [C, DH], bf16, kind="ExternalInput").ap()
    wo_d = nc.dram_tensor("wo", [DH, C], bf16, kind="ExternalInput").ap()
    sk_d = nc.dram_tensor("sk", [1, G], f32, kind="ExternalInput").ap()
    cm_d = nc.dram_tensor("cm", [128, 256], bf16, kind="ExternalInput").ap()
    vpc_d = nc.dram_tensor("vpc", [128, NKT * 128], bf16, kind="ExternalInput").ap()
    yt_d = nc.dram_tensor("yt", [C, T], f32, kind="ExternalOutput").ap()

    xt_v = xt_d.rearrange("(n p) m -> p n m", p=128)   # [128, 8, 2048]
    wq_v = wq_d.rearrange("(n p) m -> p n m", p=128)   # [128, 8, 256]
    wk_v = wk_d.rearrange("(n p) m -> p n m", p=128)
    wv_v = wv_d.rearrange("(n p) m -> p n m", p=128)
    wo_v = wo_d.rearrange("(n p) m -> p n m", p=128)   # [128, 2, 1024]
    yt_v = yt_d.rearrange("(n p) m -> p n m", p=128)   # [128, 8, 2048]

    with tile.TileContext(nc) as tc, ExitStack() as ctx:
        P = lambda name, bufs: ctx.enter_context(tc.tile_pool(name=name, bufs=bufs))
        const_p = P("const", 1)
        big_p = P("big", 1)
        p_p = P("p", PEND + 2)
        y_p = P("y", 4)
        row_p = P("row", 2)
        bcs_p = P("bcs", 2)
        ps_p = ctx.enter_context(tc.tile_pool(name="ps", bufs=3, space="PSUM"))
        o_p = ctx.enter_context(tc.tile_pool(name="o", bufs=1, space="PSUM"))

        # ---- persistent SBUF tensors ----
        xt_sb = big_p.tile([128, NCC * T], bf16, tag="xt")           # 32KB/part
        wq_sb = big_p.tile([128, NCC * DH], bf16, tag="wq")
        wk_sb = big_p.tile([128, NCC * DH], bf16, tag="wk")
        wv_sb = big_p.tile([128, NCC * DH], bf16, tag="wv")
        wo_sb = big_p.tile([128, 2 * C], bf16, tag="wo")
        # qt2: per head pair p, per q-chunk qc, head h's Q^T block of QC
        # cols at offset p*2T + qc*2QC + h*QC. Head 0 blocks keep rows
        # 64-127 zero, head 1 blocks keep rows 0-63 zero, so score matmuls
        # contract over the full 128 rows of the shared K tile.
        qt2_sb = big_p.tile([128, 2 * 2 * T], bf16, tag="qt2")
        kt_sb = big_p.tile([128, 2 * T], bf16, tag="kt")
        vp_sb = big_p.tile([128, NKT * VP_W], bf16, tag="vp")
        at_sb = big_p.tile([128, 2 * T], bf16, tag="at")             # attn^T normalized
        cm_sb = const_p.tile([128, 256], bf16, tag="cm")             # tri | tri
        skr_sb = const_p.tile([128, G], f32, tag="skr")
        esk_sb = const_p.tile([128, G], f32, tag="esk")
        eskb_sb = const_p.tile([128, G], bf16, tag="eskb")
        ones_sb = const_p.tile([128, QC], bf16, tag="ones")

        # ---- phase 0: loads + constants ----
        # weights first (K proj blocks on wk), then x in q-chunk pieces so
        # proj(0) can start after the first 512 columns of each c-chunk land
        nc.sync.dma_start(
            wk_sb[:].rearrange("p (n m) -> p n m", m=DH), wk_v[:, :, :])
        nc.sync.dma_start(
            wq_sb[:].rearrange("p (n m) -> p n m", m=DH), wq_v[:, :, :])
        nc.sync.dma_start(
            wv_sb[:].rearrange("p (n m) -> p n m", m=DH), wv_v[:, :, :])
        nc.sync.dma_start(cm_sb[:, :], cm_d[:, :])
        nc.sync.dma_start(skr_sb[0:1, :], sk_d[:, :])
        for qcl in range(NQ):
            for i in range(NCC):
                nc.sync.dma_start(
                    xt_sb[:, i * T + qcl * QC: i * T + (qcl + 1) * QC],
                    xt_v[:, i, qcl * QC:(qcl + 1) * QC])
        nc.sync.dma_start(
            wo_sb[:].rearrange("p (n m) -> p n m", m=C), wo_v[:, :, :])
        nc.gpsimd.memset(qt2_sb[:, :], 0.0)
        nc.gpsimd.memset(ones_sb[:, :], 1.0)
        nc.gpsimd.memset(eskb_sb[:, :], 0.0)
        nc.scalar.activation(esk_sb[0:1, :], skr_sb[0:1, :], AF.Exp)
        with nc.allow_low_precision(reason="bf16 sink"):
            nc.vector.tensor_copy(eskb_sb[0:1, :], esk_sb[0:1, :])
        # vp ones + zero-pad columns: cols [64,192) and [320,448) of each
        # 512-wide kt slot carry the ones columns (at 64 and 128 rel; 320
        # and 384 rel) plus the zero padding between value blocks
        vp_view = vp_sb[:].rearrange("p (k w) -> p k w", w=VP_W)
        vpc_view = vpc_d.rearrange("p (k w) -> p k w", w=128)
        nc.sync.dma_start(vp_view[:, :, 64:192], vpc_view[:, :, :])
        nc.sync.dma_start(vp_view[:, :, 320:448], vpc_view[:, :, :])

        deferred = []

        def pop():
            if deferred:
                deferred.pop(0)()

        for _ in range(reps):
            # ---- projections for one 512-wide q/k chunk, emitted piecewise
            # (yield points let attention interleave between matmul groups) ----
            def proj_steps(qc):
                for w_sb in (wk_sb, wq_sb):
                    ps = ps_p.tile([128, 2 * QC], f32, tag="ps")
                    for mt in range(2):           # head pair -> 128 d rows
                        for cig in range(2):
                            for ci in range(cig * 4, cig * 4 + 4):
                                nc.tensor.matmul(
                                    ps[:, mt * QC:(mt + 1) * QC],
                                    w_sb[:, ci * DH + mt * 128: ci * DH + (mt + 1) * 128],
                                    xt_sb[:, ci * T + qc * QC: ci * T + qc * QC + QC],
                                    start=(ci == 0), stop=(ci == NCC - 1))
                            yield
                    with nc.allow_low_precision(reason="bf16 qkv"):
                        if w_sb is wk_sb:
                            dst = kt_sb[:].rearrange("p (n m) -> p n m", m=T)[
                                :, :, qc * QC:(qc + 1) * QC]
                            nc.vector.tensor_copy(
                                dst[:, 0:1, :], ps[:, 0:QC].rearrange(
                                    "p (n m) -> p n m", m=QC))
                            nc.vector.tensor_copy(
                                dst[:, 1:2, :], ps[:, QC:2 * QC].rearrange(
                                    "p (n m) -> p n m", m=QC))
                        else:
                            # per-head halves into qt2; other half stays zero
                            for mt in range(2):
                                qb = mt * 2 * T + qc * 2 * QC
                                nc.vector.tensor_copy(
                                    qt2_sb[0:64, qb: qb + QC],
                                    ps[0:64, mt * QC:(mt + 1) * QC])
                                nc.vector.tensor_copy(
                                    qt2_sb[64:128, qb + QC: qb + 2 * QC],
                                    ps[64:128, mt * QC:(mt + 1) * QC])
                    yield
                # V natural [t, d] for tq=qc into padded vp layout
                ps = ps_p.tile([128, 2 * QC], f32, tag="ps")
                for sub in range(4):
                    tt = qc * 4 + sub
                    for ci in range(NCC):
                        nc.tensor.matmul(
                            ps[:, sub * DH:(sub + 1) * DH],
                            xt_sb[:, ci * T + tt * 128: ci * T + (tt + 1) * 128],
                            wv_sb[:, ci * DH: (ci + 1) * DH],
                            start=(ci == 0), stop=(ci == NCC - 1))
                    yield
                with nc.allow_low_precision(reason="bf16 v"):
                    for sub in range(4):
                        tt = qc * 4 + sub
                        base = tt * VP_W
                        s0 = sub * DH
                        nc.vector.tensor_copy(
                            vp_sb[:, base + 0: base + 64], ps[:, s0:s0 + 64])
                        nc.vector.tensor_copy(
                            vp_sb[:, base + 192: base + 320],
                            ps[:, s0 + 64:s0 + 192])
                        nc.vector.tensor_copy(
                            vp_sb[:, base + 448: base + 512],
                            ps[:, s0 + 192:s0 + 256])
                yield

            # ---- attention per q-chunk, software-pipelined on PE ----
            def emit_scores(p, qc, kt):
                d = kt - 4 * qc
                off = 128 * d if d > 0 else 0
                sAB = ps_p.tile([128, 2 * QC], f32, tag="ps")
                # shared K tile (A rows 0-63, B rows 64-127) as stationary,
                # zero-padded per-head Q blocks as moving operands -> full
                # 128-row contraction (sub-128 runs at half rate on HW).
                # One matmul per head: a matmul output cannot cross a PSUM
                # bank boundary.
                qb = p * 2 * T + qc * 2 * QC
                ktile = kt_sb[:, p * T + kt * 128: p * T + (kt + 1) * 128]
                nc.tensor.matmul(
                    sAB[:, off:QC], ktile,
                    qt2_sb[:, qb + off: qb + QC], start=True, stop=True)
                nc.tensor.matmul(
                    sAB[:, QC + off:2 * QC], ktile,
                    qt2_sb[:, qb + QC + off: qb + 2 * QC],
                    start=True, stop=True)
                pAB = p_p.tile([128, 2 * QC], bf16, tag="p")
                sv = sAB[:].rearrange("p (h q) -> p h q", h=2)[:, :, off:QC]
                pv = pAB[:].rearrange("p (h q) -> p h q", h=2)[:, :, off:QC]
                with nc.allow_low_precision(reason="bf16 probs"):
                    nc.scalar.activation(pv, sv, AF.Exp, scale=SCALE)
                if d >= 0:
                    mv = pAB[:].rearrange("p (h q) -> p h q", h=2)[
                        :, :, off:off + 128]
                    cmv = cm_sb[:].rearrange("p (h q) -> p h q", h=2)
                    with nc.allow_low_precision(reason="0/1 mask mult"):
                        nc.gpsimd.tensor_mul(mv, mv, cmv)
                return pAB

            def emit_pv(p, qc, kt, nkt, oAB, pAB):
                d = kt - 4 * qc
                off = 128 * d if d > 0 else 0
                hA, hB = 2 * p, 2 * p + 1
                base = kt * VP_W
                nc.tensor.matmul(
                    oAB[:, off:QC],
                    vp_sb[:, base + VP_OFF[hA]: base + VP_OFF[hA] + 128],
                    pAB[:, off:QC],
                    start=(kt == 0), stop=False,
                    skip_group_check=True)
                nc.tensor.matmul(
                    oAB[:, QC + off:2 * QC],
                    vp_sb[:, base + VP_OFF[hB]: base + VP_OFF[hB] + 128],
                    pAB[:, QC + off:2 * QC],
                    start=(kt == 0), stop=False,
                    skip_group_check=True)

            def emit_close(p, oAB):
                # sink contribution to the denominator rows closes the
                # oAB accumulation group. eskb has the values in row 0 and
                # zeros elsewhere; ones_sb is all ones, so the contraction
                # runs over the full 128 rows (full-rate on HW).
                hA, hB = 2 * p, 2 * p + 1
                nc.tensor.matmul(
                    oAB[64:65, 0:QC], eskb_sb[:, hA:hA + 1],
                    ones_sb[:, :], start=False, stop=True,
                    skip_group_check=True)
                nc.tensor.matmul(
                    oAB[0:1, QC:2 * QC], eskb_sb[:, hB:hB + 1],
                    ones_sb[:, :], start=False, stop=True,
                    skip_group_check=True)

            def emit_normalize(p, qc, oAB):
                # 1/denom rows -> SBUF, broadcast across partitions on the
                # GPSIMD engine, then scale the attention outputs on DVE.
                rc = row_p.tile([128, 2 * QC], f32, tag="rowr")
                bcs = bcs_p.tile([128, 2 * QC], f32, tag="bcs")
                # DVE cross-partition write (64->0) needs a 32-aligned dst;
                # the HW partition_broadcast only reads from partition 0
                nc.vector.reciprocal(rc[0:1, 0:QC], oAB[64:65, 0:QC])
                nc.vector.reciprocal(rc[0:1, QC:2 * QC], oAB[0:1, QC:2 * QC])
                nc.gpsimd.partition_broadcast(bcs[:, 0:QC], rc[0:1, 0:QC])
                nc.gpsimd.partition_broadcast(
                    bcs[:, QC:2 * QC], rc[0:1, QC:2 * QC])
                with nc.allow_low_precision(reason="bf16 attn out"):
                    nc.vector.tensor_mul(
                        at_sb[0:64, p * T + qc * QC: p * T + qc * QC + QC],
                        oAB[0:64, 0:QC], bcs[0:64, 0:QC])
                    nc.vector.tensor_mul(
                        at_sb[64:128, p * T + qc * QC: p * T + qc * QC + QC],
                        oAB[64:128, QC:2 * QC], bcs[64:128, QC:2 * QC])

            def make_wout(qc, cop):
                def emit():
                    ps = ps_p.tile([128, 2 * QC], f32, tag="ps")
                    for half in range(2):
                        co = cop * 2 + half
                        for j in range(2):
                            nc.tensor.matmul(
                                ps[:, half * QC:(half + 1) * QC],
                                wo_sb[:, j * C + co * 128: j * C + (co + 1) * 128],
                                at_sb[:, j * T + qc * QC: j * T + qc * QC + QC],
                                start=(j == 0), stop=(j == 1))
                    yt = y_p.tile([128, 2 * QC], f32, tag="y")
                    for half in range(2):
                        nc.vector.tensor_copy(
                            yt[:, half * QC:(half + 1) * QC],
                            ps[:, half * QC:(half + 1) * QC])
                        nc.sync.dma_start(
                            yt_v[:, cop * 2 + half, qc * QC: qc * QC + QC],
                            yt[:, half * QC:(half + 1) * QC])
                return emit

            state = {"gen": None, "first": True}

            def advance(n):
                g = state["gen"]
                if g is None:
                    return False
                for _ in range(n):
                    try:
                        next(g)
                    except StopIteration:
                        state["gen"] = None
                        return False
                return True

            state["first"] = True
            for qc in range(NQ):
                if state["first"]:
                    for i, _ in enumerate(proj_steps(qc)):
                        if i == 4:
                            pop()
                            pop()
                        elif i in (9, 14):
                            pop()
                    state["first"] = False
                state["gen"] = proj_steps(qc + 1) if qc + 1 < NQ else None
                nkt = 4 * qc + 4
                for p in range(2):
                    oAB = o_p.tile([128, 2 * QC], f32, tag="o")
                    pend = []
                    for kt in range(nkt):
                        pend.append((kt, emit_scores(p, qc, kt)))
                        if state["gen"] is not None and (p == 0 or kt % 2 == 1):
                            advance(1 if p else 2)
                        if kt >= 2 and kt % 2 == 0 and (
                                state["gen"] is None or p == 1):
                            pop()
                        if len(pend) > PEND:
                            k0, pb = pend.pop(0)
                            emit_pv(p, qc, k0, nkt, oAB, pb)
                    for k0, pb in pend:
                        emit_pv(p, qc, k0, nkt, oAB, pb)
                    emit_close(p, oAB)
                    emit_normalize(p, qc, oAB)
                for cop in range(NCC // 2):
                    deferred.append(make_wout(qc, cop))
                # drain remaining proj steps; wout pops happen inside the
                # next qc's kt loop so blocked wouts never sit in the PE
                # queue ahead of ready score matmuls
                while advance(1):
                    pass
        for fn in deferred:
            fn()
        deferred.clear()

    nc.compile()
    return nc


def make_tri_mask():
    """[128, 256] bf16: upper-tri (q>=k) pattern duplicated side by side."""
    import ml_dtypes
    kl = np.arange(128)[:, None]
    ql = np.arange(128)[None, :]
    tri = (ql >= kl).astype(np.float32)
    return np.concatenate([tri, tri], axis=1).astype(ml_dtypes.bfloat16)


def shard_inputs(x, W_Q, W_K, W_V, W_out, sink):
    import ml_dtypes
    bf16 = ml_dtypes.bfloat16
    cm = make_tri_mask()
    vpc = np.zeros((128, 128), dtype=np.float32)
    vpc[:, 0] = 1.0
    vpc[:, 64] = 1.0
    vpc = np.tile(vpc, (1, NKT)).astype(ml_dtypes.bfloat16)
    in_maps = []
    for c in range(NCORES):
        b, g = divmod(c, G)
        cols = slice(g * DH, (g + 1) * DH)
        in_maps.append({
            "xt": np.ascontiguousarray(x[b].T).astype(bf16),
            "wq": np.ascontiguousarray(W_Q[:, cols]).astype(bf16),
            "wk": np.ascontiguousarray(W_K[:, cols]).astype(bf16),
            "wv": np.ascontiguousarray(W_V[:, cols]).astype(bf16),
            "wo": np.ascontiguousarray(W_out[cols, :]).astype(bf16),
            "sk": np.ascontiguousarray(sink[g * G:(g + 1) * G][None, :]),
            "cm": cm,
            "vpc": vpc,
        })
    return in_maps


def gather_outputs(results):
    out = np.zeros((B, T, C), dtype=np.float32)
    for b in range(B):
        acc = np.zeros((C, T), dtype=np.float32)
        for g in range(G):
            acc += results[b * G + g]["yt"]
        out[b] = acc.T
    return out


_CACHE = {}


def _get_program():
    if "nc" not in _CACHE:
        _CACHE["nc"] = build_program(reps=1)
    return _CACHE["nc"]


def kernel(x, W_Q, W_K, W_V, W_out, sink):
    from concourse.bass_utils import run_bass_kernel_spmd

    x = np.asarray(x, dtype=np.float32)
    W_Q = np.asarray(W_Q, dtype=np.float32)
    W_K = np.asarray(W_K, dtype=np.float32)
    W_V = np.asarray(W_V, dtype=np.float32)
    W_out = np.asarray(W_out, dtype=np.float32)
    sink = np.asarray(sink, dtype=np.float32)

    nc = _get_program()
    in_maps = shard_inputs(x, W_Q, W_K, W_V, W_out, sink)
    res = run_bass_kernel_spmd(nc, in_maps, core_ids=list(range(NCORES)))
    return gather_outputs(res.results)
